# Initial kernel scaffold
#
"""Trainium2 Bass kernel for nn_Bilevel_35347580846320 (segment_reduce).

Computes  val = c.x + MU * sum_g ((sum_{i in g} |x_i|^2 + EPS)^(1/2))
for sorted segment_ids over N=8M elements, sharded across 8 NeuronCores.

Key idea: the output is a scalar, so per-group sums never need to be
materialized.  With sorted ids,

    sum_g sqrt(S_g + eps) = sum_i is_last[i] * sqrt(z[i] + eps)

where z is the *segmented* inclusive cumsum of x^2 (resets at group
starts) and is_last[i] = (ids[i] != ids[i+1]).  The segmented cumsum maps
directly onto the DVE TensorTensorScan instruction:

    z[t] = (A[t] * z[t-1]) + y[t],   A[t] = (ids[t] == ids[t-1]), y = x^2

Sharding: each core owns a contiguous 1M-element range; within a core each
of the 128 partitions owns a contiguous 8192-element run.  Every partition
additionally reads a W=512-element overlap window *before* its own range
(W > max group size), so any group straddling a partition/core boundary has
its full prefix inside the stream and the scan state is correct by the time
the own-region starts.  Window positions are masked out of the final
accumulation (they are owned — and counted — by the previous partition).

Per chunk (stream split into 8 chunks of 1088 columns):
  GpSimd: A   = is_equal(ids[1:], ids[:-1])                (1-port op, no
                                                            DVE contention)
  ACT:    y   = Square(x)
  DVE:    z   = scan(A, y, initial = prev chunk's last z)
  ACT:    s   = Sqrt(z + EPS)
  DVE:    STT: out=(A_next - 1) * s, accum -> -sum(is_last * s)
  DVE:    TTR: out=(x * c),          accum -> partial dot
All DVE ops here are 1-port ops -> never contend with GpSimd's shared port.
DMA via HWDGE (nc.sync) so descriptors never starve behind compute.

Each core writes out[128, 2] = (dot partials, -sqrt partials); the host
gather is a plain sum + affine epilogue (linear, like an all-reduce).
"""

import numpy as np

import concourse.bass as bass
import concourse.mybir as mybir
from concourse import tile
from concourse.bass_utils import run_bass_kernel_spmd

MU = 0.1
EPS = 1e-8
PART = 128
N_CORES = 8

# full-size geometry (hardcoded for the 8M-element problem)
F_OWN = 8192          # elements owned per partition  (128 * 8192 = 1M per core)
W = 512               # overlap window, must exceed max group size (~350)
F_CHUNK = 1088        # stream chunk width; 8 chunks of 1088 = 8704 = W + F_OWN

F32 = mybir.dt.float32
I32 = mybir.dt.int32


def build_nc(f_own=F_OWN, w=W, f_chunk=F_CHUNK):
    f_stream = f_own + w
    assert f_stream % f_chunk == 0 and w < f_chunk
    n_chunks = f_stream // f_chunk
    n_own = PART * f_own

    nc = bass.Bass()
    x_in = nc.declare_dram_parameter("xs", [n_own + w], F32, isOutput=False)
    ids_in = nc.declare_dram_parameter("idss", [n_own + w + 2], I32, isOutput=False)
    c_in = nc.declare_dram_parameter("cs", [n_own], F32, isOutput=False)
    out_h = nc.declare_dram_parameter("out", [PART, 2], F32, isOutput=True)

    Alu = mybir.AluOpType
    Act = mybir.ActivationFunctionType

    with tile.TileContext(nc) as tc:
        with (
            tc.tile_pool(name="io", bufs=3) as iop,
            tc.tile_pool(name="wk", bufs=3) as wk,
            tc.tile_pool(name="acc", bufs=1) as accp,
        ):
            wacc = accp.tile([PART, n_chunks], F32)
            dacc = accp.tile([PART, n_chunks], F32)
            z_prev = None
            for k in range(n_chunks):
                off = w if k == 0 else 0  # mask the window region (chunk 0)

                xk = iop.tile([PART, f_chunk], F32, tag="x")
                idsk = iop.tile([PART, f_chunk + 2], I32, tag="ids")
                ck = iop.tile([PART, f_chunk], F32, tag="c")
                nc.sync.dma_start(
                    out=xk[:, :],
                    in_=bass.AP(x_in, k * f_chunk, [[f_own, PART], [1, f_chunk]]),
                )
                nc.sync.dma_start(
                    out=idsk[:, :],
                    in_=bass.AP(ids_in, k * f_chunk, [[f_own, PART], [1, f_chunk + 2]]),
                )
                nc.sync.dma_start(
                    out=ck[:, : f_chunk - off],
                    in_=bass.AP(
                        c_in, k * f_chunk - w + off, [[f_own, PART], [1, f_chunk - off]]
                    ),
                )

                # A[:, m] = (ids[stream m] == ids[stream m-1]); width f_chunk+1
                # so both the scan operand (cols 0:f_chunk) and the boundary
                # mask (cols 1:f_chunk+1) come from one op.
                ak = wk.tile([PART, f_chunk + 1], F32, tag="a")
                nc.gpsimd.tensor_tensor(
                    out=ak[:, :],
                    in0=idsk[:, 1 : f_chunk + 2],
                    in1=idsk[:, 0 : f_chunk + 1],
                    op=Alu.is_equal,
                )

                yk = wk.tile([PART, f_chunk], F32, tag="y")
                nc.scalar.activation(yk[:, :], xk[:, :], Act.Square)

                zk = wk.tile([PART, f_chunk], F32, tag="z")
                initial = 0.0 if z_prev is None else z_prev[:, f_chunk - 1 : f_chunk]
                nc.vector.tensor_tensor_scan(
                    out=zk[:, :],
                    data0=ak[:, 0:f_chunk],
                    data1=yk[:, :],
                    initial=initial,
                    op0=Alu.mult,
                    op1=Alu.add,
                )
                z_prev = zk

                sk = wk.tile([PART, f_chunk], F32, tag="s")
                nc.scalar.activation(sk[:, :], zk[:, :], Act.Sqrt, bias=EPS)

                # (A_next - 1) * s accumulates -sum(is_last * sqrt(z+eps))
                junk = wk.tile([PART, f_chunk], F32, tag="junk")
                nc.vector.scalar_tensor_tensor(
                    out=junk[:, : f_chunk - off],
                    in0=ak[:, off + 1 : f_chunk + 1],
                    scalar=1.0,
                    in1=sk[:, off:f_chunk],
                    op0=Alu.subtract,
                    op1=Alu.mult,
                    accum_out=wacc[:, k : k + 1],
                )

                junk2 = wk.tile([PART, f_chunk], F32, tag="junk2")
                nc.vector.tensor_tensor_reduce(
                    out=junk2[:, : f_chunk - off],
                    in0=xk[:, off:f_chunk],
                    in1=ck[:, : f_chunk - off],
                    scale=1.0,
                    scalar=0.0,
                    op0=Alu.mult,
                    op1=Alu.add,
                    accum_out=dacc[:, k : k + 1],
                )

            fin = accp.tile([PART, 2], F32)
            nc.vector.tensor_reduce(
                out=fin[:, 0:1], in_=dacc[:, :], axis=mybir.AxisListType.X, op=Alu.add
            )
            nc.vector.tensor_reduce(
                out=fin[:, 1:2], in_=wacc[:, :], axis=mybir.AxisListType.X, op=Alu.add
            )
            nc.sync.dma_start(out=out_h[:, :], in_=fin[:, :])
    return nc


def make_in_maps(x, c, segment_ids, f_own=F_OWN, w=W, n_cores=N_CORES):
    """Slice the full inputs into per-core overlapping shards (pure indexing)."""
    x = np.ascontiguousarray(x, dtype=np.float32)
    c = np.ascontiguousarray(c, dtype=np.float32)
    ids = np.ascontiguousarray(segment_ids, dtype=np.int32)
    n = x.shape[0]
    n_own = PART * f_own
    assert n == n_cores * n_own

    x_pad = np.concatenate([np.zeros(w, np.float32), x])
    ids_pad = np.concatenate(
        [np.full(w + 1, -1, np.int32), ids, np.full(1, -2, np.int32)]
    )
    in_maps = []
    for m in range(n_cores):
        s = m * n_own
        in_maps.append(
            {
                "xs": x_pad[s : s + n_own + w].copy(),
                "idss": ids_pad[s : s + n_own + w + 2].copy(),
                "cs": c[s : s + n_own].copy(),
            }
        )
    return in_maps


def gather(outs):
    """outs: [n_cores, 128, 2] partials -> scalar result."""
    outs = np.asarray(outs, dtype=np.float64)
    dot = outs[..., 0].sum()
    sqrt_sum = -outs[..., 1].sum()
    return np.float32(dot + MU * sqrt_sum)


_NC_CACHE = {}


def kernel(x, c, segment_ids, n_groups=None, **run_kwargs):
    key = "full"
    if key not in _NC_CACHE:
        _NC_CACHE[key] = build_nc()
    nc = _NC_CACHE[key]
    in_maps = make_in_maps(x, c, segment_ids)
    res = run_bass_kernel_spmd(
        nc, in_maps, core_ids=list(range(N_CORES)), **run_kwargs
    )
    outs = np.stack([r["out"] for r in res.results])
    result = gather(outs)
    kernel.last_results = res
    return result


# revision 13
# speedup vs baseline: 2.0221x; 2.0221x over previous
"""Trainium2 Bass kernel for nn_Bilevel_35347580846320 (segment_reduce).

Computes  val = c.x + MU * sum_g ((sum_{i in g} |x_i|^2 + EPS)^(1/2))
for sorted segment_ids over N=8M elements, sharded across 8 NeuronCores.

Key idea: the output is a scalar, so per-group sums never need to be
materialized.  With sorted ids,

    sum_g sqrt(S_g + eps) = sum_i is_last[i] * sqrt(z[i] + eps)

where z is the *segmented* inclusive cumsum of x^2 (resets at group
starts) and is_last[i] = (ids[i] != ids[i+1]).  The segmented cumsum maps
directly onto the DVE TensorTensorScan instruction:

    z[t] = (A[t] * z[t-1]) + y[t],   A[t] = (ids[t] == ids[t-1]), y = x^2

Sharding: each core owns a contiguous 1M-element range; within a core each
of the 128 partitions owns a contiguous 8192-element run.  Every partition
additionally reads a W=512-element overlap window *before* its own range
(W > max group size), so any group straddling a partition/core boundary has
its full prefix inside the stream and the scan state is correct by the time
the own-region starts.  Window positions are masked out of the final
accumulation (they are owned — and counted — by the previous partition).

Per chunk (stream split into 8 chunks of 1088 columns):
  GpSimd: A   = is_equal(ids[1:], ids[:-1])                (1-port op, no
                                                            DVE contention)
  ACT:    y   = Square(x)
  DVE:    z   = scan(A, y, initial = prev chunk's last z)
  ACT:    s   = Sqrt(z + EPS)
  DVE:    STT: out=(A_next - 1) * s, accum -> -sum(is_last * s)
  DVE:    TTR: out=(x * c),          accum -> partial dot
All DVE ops here are 1-port ops -> never contend with GpSimd's shared port.
DMA via HWDGE (nc.sync) so descriptors never starve behind compute.

Each core writes out[128, 2] = (dot partials, -sqrt partials); the host
gather is a plain sum + affine epilogue (linear, like an all-reduce).
"""

import numpy as np

import concourse.bacc as bacc
import concourse.bass as bass
import concourse.mybir as mybir
from concourse import tile
from concourse.bass_utils import run_bass_kernel_spmd

MU = 0.1
EPS = 1e-8
PART = 128
N_CORES = 8

# full-size geometry (hardcoded for the 8M-element problem)
# 8M / (8 cores * 128 partitions) = 7812.5, so the host pads the global
# array with 512 trailing elements (x=c=0, ids=PAD_ID) to 1024*7813.
F_OWN = 7813          # elements owned per partition
W = 512               # overlap window, must exceed max group size (~350)
F_CHUNK = 925         # stream chunk width; 9 chunks of 925 = 8325 = W + F_OWN
PAD_ID = 0x3FFFFFFF   # segment id for pad elements (y=0 there, so harmless)

F32 = mybir.dt.float32
I32 = mybir.dt.int32


def build_nc(f_own=F_OWN, w=W, f_chunk=F_CHUNK, repeat=1):
    f_stream = f_own + w
    assert f_stream % f_chunk == 0 and w < f_chunk
    n_chunks = f_stream // f_chunk
    n_own = PART * f_own

    nc = bacc.Bacc()
    x_in = nc.declare_dram_parameter("xs", [n_own + w], F32, isOutput=False)
    ids_in = nc.declare_dram_parameter("idss", [n_own + w + 2], I32, isOutput=False)
    c_in = nc.declare_dram_parameter("cs", [n_own], F32, isOutput=False)
    out_h = nc.declare_dram_parameter("out", [PART, 2], F32, isOutput=True)

    Alu = mybir.AluOpType
    Act = mybir.ActivationFunctionType

    with tile.TileContext(nc) as tc:
        with (
            tc.tile_pool(name="io", bufs=3) as iop,
            tc.tile_pool(name="wk", bufs=3) as wk,
            tc.tile_pool(name="acc", bufs=1) as accp,
        ):
            wacc = accp.tile([PART, n_chunks], F32)
            dacc = accp.tile([PART, n_chunks], F32)
            eps_t = accp.tile([PART, 1], F32)
            nc.gpsimd.memset(eps_t[:, :], EPS)

            def body():
                z_prev = None
                for k in range(n_chunks):
                    off = w if k == 0 else 0  # mask the window region (chunk 0)

                    xk = iop.tile([PART, f_chunk], F32, tag="x")
                    idsk = iop.tile([PART, f_chunk + 2], I32, tag="ids")
                    ck = iop.tile([PART, f_chunk], F32, tag="c")
                    nc.sync.dma_start(
                        out=xk[:, :],
                        in_=bass.AP(x_in, k * f_chunk, [[f_own, PART], [1, f_chunk]]),
                    )
                    nc.sync.dma_start(
                        out=idsk[:, :],
                        in_=bass.AP(
                            ids_in, k * f_chunk, [[f_own, PART], [1, f_chunk + 2]]
                        ),
                    )
                    nc.sync.dma_start(
                        out=ck[:, : f_chunk - off],
                        in_=bass.AP(
                            c_in,
                            k * f_chunk - w + off,
                            [[f_own, PART], [1, f_chunk - off]],
                        ),
                    )

                    # A[:, m] = (ids[stream m] == ids[stream m-1]); width
                    # f_chunk+1 so both the scan operand (cols 0:f_chunk) and
                    # the boundary mask (cols 1:f_chunk+1) come from one op.
                    ak = wk.tile([PART, f_chunk + 1], F32, tag="a")
                    nc.vector.tensor_tensor(
                        out=ak[:, :],
                        in0=idsk[:, 1 : f_chunk + 2],
                        in1=idsk[:, 0 : f_chunk + 1],
                        op=Alu.is_equal,
                    )

                    yk = wk.tile([PART, f_chunk], F32, tag="y")
                    nc.scalar.activation(yk[:, :], xk[:, :], Act.Square)

                    zk = wk.tile([PART, f_chunk], F32, tag="z")
                    initial = (
                        0.0 if z_prev is None else z_prev[:, f_chunk - 1 : f_chunk]
                    )
                    nc.vector.tensor_tensor_scan(
                        out=zk[:, :],
                        data0=ak[:, 0:f_chunk],
                        data1=yk[:, :],
                        initial=initial,
                        op0=Alu.mult,
                        op1=Alu.add,
                    )
                    z_prev = zk

                    sk = wk.tile([PART, f_chunk], F32, tag="s")
                    nc.scalar.activation(
                        sk[:, :], zk[:, :], Act.Sqrt, bias=eps_t[:, :]
                    )

                    # (A_next - 1) * s accumulates -sum(is_last * sqrt(z+eps))
                    junk = wk.tile([PART, f_chunk], F32, tag="junk")
                    nc.vector.scalar_tensor_tensor(
                        out=junk[:, : f_chunk - off],
                        in0=ak[:, off + 1 : f_chunk + 1],
                        scalar=1.0,
                        in1=sk[:, off:f_chunk],
                        op0=Alu.subtract,
                        op1=Alu.mult,
                        accum_out=wacc[:, k : k + 1],
                    )

                    junk2 = wk.tile([PART, f_chunk], F32, tag="junk2")
                    nc.vector.scalar_tensor_tensor(
                        out=junk2[:, : f_chunk - off],
                        in0=xk[:, off:f_chunk],
                        scalar=1.0,
                        in1=ck[:, : f_chunk - off],
                        op0=Alu.mult,
                        op1=Alu.mult,
                        accum_out=dacc[:, k : k + 1],
                    )

                fin = accp.tile([PART, 2], F32, tag="fin")
                nc.vector.tensor_reduce(
                    out=fin[:, 0:1],
                    in_=dacc[:, :],
                    axis=mybir.AxisListType.X,
                    op=Alu.add,
                )
                nc.vector.tensor_reduce(
                    out=fin[:, 1:2],
                    in_=wacc[:, :],
                    axis=mybir.AxisListType.X,
                    op=Alu.add,
                )
                nc.sync.dma_start(out=out_h[:, :], in_=fin[:, :])

            if repeat > 1:
                with tc.For_i(0, repeat, 1):
                    body()
            else:
                body()
    nc.compile()
    return nc


def make_in_maps(x, c, segment_ids, f_own=F_OWN, w=W, n_cores=N_CORES):
    """Slice the full inputs into per-core overlapping shards (pure indexing)."""
    x = np.ascontiguousarray(x, dtype=np.float32)
    c = np.ascontiguousarray(c, dtype=np.float32)
    ids = np.ascontiguousarray(segment_ids, dtype=np.int32)
    n = x.shape[0]
    n_own = PART * f_own
    n_padded = n_cores * n_own
    pad = n_padded - n
    assert pad >= 0
    # Trailing pad: x=c=0 so dot/z contributions vanish; one constant PAD_ID
    # group whose segmented sums are exactly 0 contributes MU*sqrt(EPS)=1e-5
    # (negligible vs the ~5e4 result).
    if pad:
        x = np.concatenate([x, np.zeros(pad, np.float32)])
        c = np.concatenate([c, np.zeros(pad, np.float32)])
        ids = np.concatenate([ids, np.full(pad, PAD_ID, np.int32)])

    x_pad = np.concatenate([np.zeros(w, np.float32), x])
    ids_pad = np.concatenate(
        [np.full(w + 1, -1, np.int32), ids, np.full(1, -2, np.int32)]
    )
    in_maps = []
    for m in range(n_cores):
        s = m * n_own
        in_maps.append(
            {
                "xs": x_pad[s : s + n_own + w].copy(),
                "idss": ids_pad[s : s + n_own + w + 2].copy(),
                "cs": c[s : s + n_own].copy(),
            }
        )
    return in_maps


def gather(outs):
    """outs: [n_cores, 128, 2] partials -> scalar result."""
    outs = np.asarray(outs, dtype=np.float64)
    dot = outs[..., 0].sum()
    sqrt_sum = -outs[..., 1].sum()
    return np.float32(dot + MU * sqrt_sum)


_NC_CACHE = {}


def kernel(x, c, segment_ids, n_groups=None, **run_kwargs):
    key = "full"
    if key not in _NC_CACHE:
        _NC_CACHE[key] = build_nc()
    nc = _NC_CACHE[key]
    in_maps = make_in_maps(x, c, segment_ids)
    res = run_bass_kernel_spmd(
        nc, in_maps, core_ids=list(range(N_CORES)), **run_kwargs
    )
    outs = np.stack([r["out"] for r in res.results])
    result = gather(outs)
    kernel.last_results = res
    return result
